# revision 8
# baseline (speedup 1.0000x reference)
"""CapsuleLayer routing kernel for Trainium2, 8 NeuronCores.

Math (per reference):
    u[b,i,n,d] = sum_k W[0,i,n,d,k] * x[b,i,k]
    s[b,i,n]   = sum_d u[b,i,n,d]
    b1 = s/32 ; c1 = softmax_n(b1) ; b2 = b1 + c1*s ; c2 = softmax_n(b2)
    out[b,n,d] = squash(sum_i c2[b,i,n] * u[b,i,n,d])

Routing is fully local per (b,i); only the final i-sum couples cores.

Sharding: tensor-parallel over IN_CAPS (2048/8 = 256 i per core). Each core
returns its partial pre-squash sum [32, 512]; the host adds the 8 partials
(the unshard for a reduction-sharded output) and applies squash.

Device layout (host-prepared, per core), free dim = (d, n) so that the
c2 broadcast is innermost-dense (DVE 2x mode) and the g-tree is dense:
  wt  [32, 128, 512] f32: wt[m, 64*h + 16*j + k, 32*d + n] = W[0, i0+4*(2m+h)+j, n, d, k]
  xbd [32, 128, 128] f32: block-diagonal stationary tiles;
      xbd[m, 64*h + 16*j + k, 32*j + b] = x[b, i0+4*(2m+h)+j, k]
  matmul per group g=2m+h: lhsT = xbd[64h:, 128m:] (K=64 = 4j*16k,
      M=128 = 4j*32b), rhs = wt[64h:, m-slice] (N=512 = 16d*32n)
      -> u_g[32j+b, 32*d + n] in PSUM.

Pipeline per batch of 8 groups: 8 matmuls -> evict (ACT/DVE split, bf16)
-> s d-tree (GpSimd) -> routing softmax ops (DVE+ACT) -> c2-mult (DVE).
Then a dense g-tree, partition fold, DMA out.
"""

import os
import sys

import numpy as np

sys.path.insert(0, "/opt/trn_rl_repo")

import concourse.bacc as bacc
import concourse.bass as bass
import concourse.mybir as mybir
import concourse.tile as tile
from concourse.bass_utils import run_bass_kernel_spmd

B = 32
IN_CAPS = 2048
IN_DIM = 16
NUM_CAPS = 32
DIM_CAPS = 16
NCORES = 8
NI = IN_CAPS // NCORES  # 256
G = NI // 4  # 64 groups of 4 i's
M = G // 2  # 32 pair-tiles
ND = NUM_CAPS * DIM_CAPS  # 512
GB = 8  # groups per pipeline batch
NB = G // GB  # 8 batches
EPS = 1e-7

F32 = mybir.dt.float32
BF16 = mybir.dt.bfloat16
X = mybir.AxisListType.X
ADD = mybir.AluOpType.add
MULT = mybir.AluOpType.mult

LAST_RESULTS = None
_NC = None


def _routing_batch(nc, pools, S, c2, b):
    """Softmax routing for one batch of GB groups: S[:, b] -> c2[:, b]."""
    small = pools
    gn = GB * NUM_CAPS  # 256
    sl = slice(b * gn, (b + 1) * gn)
    Sb = S[:, sl]
    E1 = small.tile([128, gn], BF16, tag="E1")
    nc.scalar.activation(E1[:], Sb, mybir.ActivationFunctionType.Exp, scale=1.0 / 32.0)
    Z1 = small.tile([128, GB], F32, tag="Z1")
    nc.vector.tensor_reduce(
        out=Z1[:], in_=E1[:].rearrange("p (g n) -> p g n", n=NUM_CAPS), axis=X, op=ADD
    )
    R1 = small.tile([128, GB], F32, tag="R1")
    nc.vector.reciprocal(R1[:], Z1[:])
    c1 = small.tile([128, gn], BF16, tag="c1")
    nc.vector.tensor_mul(
        c1[:].rearrange("p (g n) -> p g n", n=NUM_CAPS),
        E1[:].rearrange("p (g n) -> p g n", n=NUM_CAPS),
        R1[:].unsqueeze(2).broadcast_to((128, GB, NUM_CAPS)),
    )
    B2 = small.tile([128, gn], F32, tag="B2")
    nc.vector.scalar_tensor_tensor(
        out=B2[:], in0=c1[:], scalar=1.0 / 32.0, in1=Sb, op0=ADD, op1=MULT
    )
    E2 = small.tile([128, gn], BF16, tag="E2")
    nc.scalar.activation(E2[:], B2[:], mybir.ActivationFunctionType.Exp)
    Z2 = small.tile([128, GB], F32, tag="Z2")
    nc.vector.tensor_reduce(
        out=Z2[:], in_=E2[:].rearrange("p (g n) -> p g n", n=NUM_CAPS), axis=X, op=ADD
    )
    R2 = small.tile([128, GB], F32, tag="R2")
    nc.vector.reciprocal(R2[:], Z2[:])
    nc.vector.tensor_mul(
        c2[:, sl].rearrange("p (g n) -> p g n", n=NUM_CAPS),
        E2[:].rearrange("p (g n) -> p g n", n=NUM_CAPS),
        R2[:].unsqueeze(2).broadcast_to((128, GB, NUM_CAPS)),
    )


def _kernel_body(tc):
    nc = tc.nc
    xbd_d = nc.dram_tensor("xbd", [M, 128, 128], F32, kind="ExternalInput").ap()
    wt_d = nc.dram_tensor("wt", [M, 128, ND], F32, kind="ExternalInput").ap()
    out_d = nc.dram_tensor("partial", [B, ND], F32, kind="ExternalOutput").ap()

    from contextlib import ExitStack

    with ExitStack() as ctx:
        const_pool = ctx.enter_context(tc.tile_pool(name="const", bufs=1))
        psum = ctx.enter_context(tc.tile_pool(name="ps", bufs=8, space="PSUM"))
        big = ctx.enter_context(tc.tile_pool(name="big", bufs=1))
        small = ctx.enter_context(tc.tile_pool(name="small", bufs=2))

        # Stationary block-diag x tiles: [128, M*128] bf16 (cast during DMA).
        xbd_sb = const_pool.tile([128, M * 128], BF16)
        nc.gpsimd.dma_start(
            out=xbd_sb[:].rearrange("p (m c) -> p m c", c=128),
            in_=xbd_d.rearrange("m p c -> p m c"),
        )

        # All W resident as one bf16 tile, loaded in 4 coarse cast-DMAs.
        wt_sb = const_pool.tile([128, M * ND], BF16)
        mper = M // 4
        for q in range(4):
            nc.gpsimd.dma_start(
                out=wt_sb[:, q * mper * ND : (q + 1) * mper * ND].rearrange(
                    "p (m f) -> p m f", f=ND
                ),
                in_=wt_d.rearrange("m p f -> p m f")[:, q * mper : (q + 1) * mper, :],
            )

        U = big.tile([128, G * ND], BF16)  # evicted u, (g, d, n), 64KB/part
        S = small.tile([128, G * NUM_CAPS], F32, tag="S")
        c2 = small.tile([128, G * NUM_CAPS], BF16, tag="c2")
        Ta = big.tile([128, GB * 8 * NUM_CAPS], BF16)  # s-tree scratch [128, 2048]
        Tb = big.tile([128, GB * 4 * NUM_CAPS], BF16)  # s-tree scratch [128, 1024]

        for b in range(NB):
            # --- matmuls + evictions for GB groups ---
            for gl in range(GB):
                g = b * GB + gl
                m, h = divmod(g, 2)
                u_ps = psum.tile([128, ND], F32)
                nc.tensor.matmul(
                    u_ps[:],
                    lhsT=xbd_sb[64 * h : 64 * (h + 1), m * 128 : (m + 1) * 128],
                    rhs=wt_sb[64 * h : 64 * (h + 1), m * ND : (m + 1) * ND],
                    start=True,
                    stop=True,
                )
                dst = U[:, g * ND : (g + 1) * ND]
                if gl < 5:
                    nc.scalar.copy(dst, u_ps[:])
                else:
                    nc.vector.tensor_scalar_mul(dst, u_ps[:], 1.0)

            # --- s = sum_d u: d-tree on GpSimd (strided in, dense out) ---
            Ub = U[:, b * GB * ND : (b + 1) * GB * ND].rearrange(
                "p (g d n) -> p g d n", d=DIM_CAPS, n=NUM_CAPS
            )
            # L1: Ta[(g,8d,n)] = U[:, :, :8, :] + U[:, :, 8:, :]
            Ta1 = Ta[:].rearrange("p (g d n) -> p g d n", d=8, n=NUM_CAPS)
            nc.gpsimd.tensor_add(Ta1, Ub[:, :, 0:8, :], Ub[:, :, 8:16, :])
            # L2: Tb[(g,4d,n)] = Ta[:, :, :4, :] + Ta[:, :, 4:, :]
            Tb1 = Tb[:].rearrange("p (g d n) -> p g d n", d=4, n=NUM_CAPS)
            nc.gpsimd.tensor_add(Tb1, Ta1[:, :, 0:4, :], Ta1[:, :, 4:8, :])
            # L3: Ta[(g,2d,n)] = Tb[:, :, :2, :] + Tb[:, :, 2:, :]
            Ta2 = Ta[:, : GB * 2 * NUM_CAPS].rearrange(
                "p (g d n) -> p g d n", d=2, n=NUM_CAPS
            )
            nc.gpsimd.tensor_add(Ta2, Tb1[:, :, 0:2, :], Tb1[:, :, 2:4, :])
            # L4 -> S slice (f32)
            nc.gpsimd.tensor_add(
                S[:, b * GB * NUM_CAPS : (b + 1) * GB * NUM_CAPS].rearrange(
                    "p (g n) -> p g n", n=NUM_CAPS
                ),
                Ta2[:, :, 0, :],
                Ta2[:, :, 1, :],
            )

            # --- routing for this batch -> c2 slice ---
            _routing_batch(nc, small, S, c2, b)

            # --- apply c2 to U batch (innermost-dense broadcast, 2x) ---
            nc.vector.tensor_mul(
                Ub,
                Ub,
                c2[:, b * GB * NUM_CAPS : (b + 1) * GB * NUM_CAPS]
                .rearrange("p (g n) -> p g n", n=NUM_CAPS)
                .unsqueeze(2)
                .broadcast_to((128, GB, DIM_CAPS, NUM_CAPS)),
            )

        # --- dense tree-reduce over g (g outermost) ---
        w = G
        while w > 1:
            hw_ = w // 2
            half = U[:, : hw_ * ND]
            other = U[:, hw_ * ND : w * ND]
            if w > 8:
                # split across DVE / GpSimd by free range (2:1)
                cut = (2 * hw_ // 3) * ND
                nc.vector.tensor_add(half[:, :cut], half[:, :cut], other[:, :cut])
                nc.gpsimd.tensor_add(half[:, cut:], half[:, cut:], other[:, cut:])
            else:
                nc.vector.tensor_add(half, half, other)
            w = hw_

        # Fold the 4 j-blocks of partitions: partial[b] = sum_j ACC[32j+b].
        tshift = small.tile([32, 3 * ND], BF16, tag="tshift")
        for q in range(3):
            nc.sync.dma_start(
                out=tshift[:, q * ND : (q + 1) * ND],
                in_=U[32 * (q + 1) : 32 * (q + 2), :ND],
            )
        P0 = small.tile([32, ND], F32, tag="P0")
        nc.vector.tensor_add(P0[:], U[0:32, :ND], tshift[:, 0:ND])
        nc.vector.tensor_add(P0[:], P0[:], tshift[:, ND : 2 * ND])
        nc.vector.tensor_add(P0[:], P0[:], tshift[:, 2 * ND : 3 * ND])

        nc.sync.dma_start(out=out_d[:], in_=P0[:])


def build_nc():
    nc = bacc.Bacc(
        "TRN2",
        target_bir_lowering=False,
        debug=False,
        enable_asserts=False,
        num_devices=NCORES,
    )
    with tile.TileContext(nc) as tc:
        _kernel_body(tc)
    nc.compile()
    return nc


def _get_nc():
    global _NC
    if _NC is None:
        _NC = build_nc()
    return _NC


def prep_core_inputs(x, W, c):
    """Host-side shard + layout prep for core c (pure relayout, no math)."""
    sl = slice(c * NI, (c + 1) * NI)
    xt = np.ascontiguousarray(np.transpose(x[:, sl, :], (1, 2, 0)))  # [NI, k, b]
    xt = xt.reshape(G, 4, IN_DIM, B)
    xbd = np.zeros((G, 64, 128), np.float32)
    for j in range(4):
        xbd[:, 16 * j : 16 * (j + 1), 32 * j : 32 * (j + 1)] = xt[:, j]
    xbd = np.ascontiguousarray(xbd.reshape(M, 128, 128))
    ws = W[0, sl]  # [NI, n, d, k]
    wt = np.ascontiguousarray(np.transpose(ws, (0, 3, 2, 1))).reshape(G, 64, ND)
    wt = np.ascontiguousarray(wt.reshape(M, 128, ND))  # [m, (h,j,k), (d,n)]
    return {"xbd": xbd, "wt": wt}


def kernel(x, W):
    global LAST_RESULTS
    x = np.asarray(x, dtype=np.float32)
    W = np.asarray(W, dtype=np.float32)
    in_maps = [prep_core_inputs(x, W, c) for c in range(NCORES)]
    res = run_bass_kernel_spmd(_get_nc(), in_maps, core_ids=list(range(NCORES)))
    LAST_RESULTS = res
    v = np.sum([r["partial"] for r in res.results], axis=0, dtype=np.float32)
    # device layout is [b, (d, n)] -> [b, n, d]
    v = v.reshape(B, DIM_CAPS, NUM_CAPS).transpose(0, 2, 1)
    z = np.sum(v * v, axis=-1, keepdims=True, dtype=np.float32)
    scale = z / (1.0 + z) / np.sqrt(z + EPS)
    return np.ascontiguousarray((scale * v).astype(np.float32))
